# revision 12
# baseline (speedup 1.0000x reference)
"""EquivariantProjectorViaSchur — TRN2 Bass kernel (8 NeuronCores, SPMD).

Math (per 64x64 channel block B of W):
    V   = U_y^T B U_x
    P   = mask o V, with matched 2x2 rotation blocks symmetrized
    out = U_y P U_x^T
Pipeline (per core, c_in block-column shard of W, all-bf16 datapath with
fp32 PSUM accumulation; rel-err budget is 2e-2, bf16 lands ~5e-3):
    A:  T1T = (U_y[:,tau]^T B)^T per 128-row stripe: W-chunk-stationary
        bf16 matmuls (FWL weight loads), moving LYtau = kron(I2, U_y[:,tau]).
    B:  Z[:, o in g] = XG_g T1T[:, o in g] (+ J-term).  tau orders each
        rotation group's columns [evens | odds] so the J-term is matmuls on
        contiguous swapped column ranges with +/-XJ stationaries -- no
        sigma-prep DVE copies.  Stationary-major emission amortizes LDW.
    C:  PE transpose of Z (bf16).
    D:  out = kron(I2, U_y_tau^T)-contraction of Z^T.
W ships to the device as bf16 (host cast) and the output returns as bf16
(host upcast) -- halves both DMA directions; error stays well in budget.
Sharding: c_in block-columns -- core i owns W[:, i*768:(i+1)*768]; the tiny
U/mask-derived factor matrices are replicated (precomputed host-side).
"""
import contextlib
import time

import numpy as np

import concourse.bass as bass
import concourse.tile as tile
import concourse.mybir as mybir
from concourse.tile import ScopedClock

F32 = mybir.dt.float32
F32R = mybir.dt.float32r
BF16 = mybir.dt.bfloat16

O = 64
NCH = 2               # chunks of 24 stripes (48 c_out blocks)
NSTR = 24             # 128-row stripes per chunk
NQ = 3                # quads (cc-pairs) per core shard
NCORE = 8
CSH = 768             # columns per core shard


# ---------------------------------------------------------------------------
# workarounds for this toolchain
# ---------------------------------------------------------------------------
def _patched_drain_and_barrier(self, tick_clock, wait_clock):
    # this walrus build rejects >1 sem-wait on a Drain: split the tail waits
    drain_inst = self.nc.sync.drain()
    wait_clock.add_sem_waits(drain_inst.ins,
                             ScopedClock({None: tick_clock.global_clock}))
    si = drain_inst.ins.sync_info
    waits = list(si.on_wait) if si is not None else []
    if len(waits) > 1:
        drain_inst.ins.sync_info = mybir.SyncInfo(
            on_wait=waits[:1], on_update=list(si.on_update))
        for i in range(1, len(waits)):
            d2 = self.nc.sync.drain()
            d2.ins.sync_info = mybir.SyncInfo(on_wait=[waits[i]], on_update=[])
    self.nc.all_engine_barrier()
    assert self.sems is not None
    popped = self.nc._tile_sem_poison_stack.pop()
    assert popped is self._sem_poison
    self.nc.clear_and_free_semaphores(list(self.sems.allocated().values()))
    self.nc.all_engine_barrier()


tile.TileContext._drain_and_barrier = _patched_drain_and_barrier


def cap_sync_waits(nc):
    """walrus codegen allows only 1 sem-wait per instruction struct here;
    carry the excess on NoOps inserted just before (same engine/point)."""
    for f in nc.m.functions:
        for blk in f.blocks:
            insts = list(blk.instructions)
            out = []
            ctr = 0
            for ins in insts:
                si = ins.sync_info
                waits = list(si.on_wait) if si is not None else []
                if len(waits) > 1:
                    for i in range(len(waits) - 1):
                        n = mybir.InstNoOp(name=f"{ins.name}_w{ctr}",
                                           ins=[], outs=[])
                        ctr += 1
                        n.engine = ins.engine
                        n.sync_info = mybir.SyncInfo(on_wait=[waits[i]],
                                                     on_update=[])
                        out.append(n)
                    ins.sync_info = mybir.SyncInfo(
                        on_wait=waits[-1:], on_update=list(si.on_update))
                out.append(ins)
            blk.instructions = out


# ---------------------------------------------------------------------------
# host-side precompute of the replicated factor matrices
# ---------------------------------------------------------------------------
def host_precompute(U_y, U_x, mask, block_rows, block_cols):
    rows = np.asarray(block_rows); cols = np.asarray(block_cols)
    mask = np.asarray(mask)
    U_y64 = np.asarray(U_y, np.float64); U_x64 = np.asarray(U_x, np.float64)
    r_rot = set(int(x) for x in rows.tolist())
    nqd = len(rows) // 4
    for t in range(nqd):
        r = rows[4 * t:4 * t + 4]; c = cols[4 * t:4 * t + 4]
        assert mask[r, c].all()
        assert r[0] == r[1] and r[2] == r[3] and r[2] == r[0] + 1 and r[0] % 2 == 0
        assert c[0] == c[2] and c[1] == c[3] and c[1] == c[0] + 1 and c[0] % 2 == 0
    groups, seen = [], np.zeros(O, bool)
    for k in range(O):
        if seen[k]:
            continue
        mem = np.where(mask[k] > 0)[0]
        assert (mask[np.ix_(mem, mem)] > 0).all()
        for m in mem:
            seen[m] = True
        groups.append(mem)
    pi = np.where(np.arange(O) % 2 == 0, 1.0, -1.0)
    eye2 = np.eye(2)

    # tau: relabel the transformed-y axis so each rotation group's columns
    # are [evens | odds] (position p and p+len/2 are XOR-partners) and each
    # diag group is contiguous.  R groups first, then D groups.
    tau = []
    ginfo = []
    r_groups = [m for m in groups if int(m[0]) in r_rot]
    d_groups = [m for m in groups if int(m[0]) not in r_rot]
    for mem in r_groups:
        evens = [int(m) for m in mem if m % 2 == 0]
        odds = [e ^ 1 for e in evens]
        assert sorted(evens + odds) == sorted(int(m) for m in mem)
        pos0 = len(tau)
        tau += evens + odds
        ginfo.append(dict(is_R=True, pos0=pos0, half=len(evens)))
    for mem in d_groups:
        pos0 = len(tau)
        tau += [int(m) for m in mem]
        ginfo.append(dict(is_R=False, pos0=pos0, size=len(mem)))
    assert len(tau) == O and sorted(tau) == list(range(O))
    tau = np.asarray(tau, np.int64)
    U_yt = U_y64[:, tau]

    mats = []

    def add(m64):
        mats.append(np.kron(eye2, m64))
        return len(mats) - 1

    meta = dict(
        i_ly=add(U_yt),          # phase A moving
        i_ls4=add(U_yt.T),       # phase D stationary
        i_id=add(np.eye(O)),     # phase C transpose moving
    )
    for g, mem in zip(ginfo, r_groups + d_groups):
        s = 0.5 if g["is_R"] else 1.0
        a = np.zeros(O); a[mem] = 1.0
        XG = s * (U_x64 @ np.diag(a) @ U_x64.T)          # symmetric
        g["gi"] = add(XG)
        if g["is_R"]:
            XJ = np.zeros((O, O))
            for k in mem:
                XJ += 0.5 * pi[k] * np.outer(U_x64[:, k], U_x64[:, k ^ 1])
            g["ji"] = add(XJ.T)
            g["jni"] = add(-XJ.T)
    cst = np.concatenate(mats, axis=1)                   # [128, n*128] f64
    return cst, ginfo, meta


class _EvacBalancer:
    """Greedy ACT/DVE assignment for PSUM->SBUF copies.  DVE runs 2x when
    every operand is a packed 2-byte dtype."""
    def __init__(self, nc):
        self.nc = nc
        self.t_act = 0.0
        self.t_dve = 0.0

    def copy(self, dst, src):
        fd = src.free_size()
        both16 = (mybir.dt.size(src.dtype) == 2
                  and mybir.dt.size(dst.dtype) == 2)
        c_act = (172.0 + fd) / 1.2
        c_dve = (120.0 + (fd * 0.5 if both16 else fd)) / 0.96
        if self.t_act + c_act <= self.t_dve + c_dve:
            self.t_act += c_act
            return self.nc.scalar.copy(dst, src)
        else:
            self.t_dve += c_dve
            return self.nc.vector.tensor_copy(dst, src)


# ---------------------------------------------------------------------------
# device kernel (one program, SPMD over 8 cores)
# ---------------------------------------------------------------------------
def build_kernel(n_mats, ginfo, meta):
    nc = bass.Bass("TRN2", target_bir_lowering=False, debug=False,
                   num_devices=1)
    w = nc.dram_tensor("w", [6144, CSH], BF16, kind="ExternalInput").ap()
    cst = nc.dram_tensor("cst", [128, n_mats * 128], BF16,
                         kind="ExternalInput").ap()
    out = nc.dram_tensor("out", [NQ, 12, 128, 1024], BF16,
                         kind="ExternalOutput").ap()

    with tile.TileContext(nc) as tc:
        ctx = contextlib.ExitStack()
        with ctx:
            ev = _EvacBalancer(nc)
            cs_p = ctx.enter_context(tc.tile_pool(name="cs", bufs=1))
            wch_p = ctx.enter_context(tc.tile_pool(name="wch", bufs=6))
            t1_p = ctx.enter_context(tc.tile_pool(name="t1", bufs=1))
            zsb_p = ctx.enter_context(tc.tile_pool(name="zsb", bufs=1))
            zt_p = ctx.enter_context(tc.tile_pool(name="zt", bufs=2))
            osb_p = ctx.enter_context(tc.tile_pool(name="osb", bufs=4))
            ps_a = ctx.enter_context(
                tc.tile_pool(name="ps_a", bufs=3, space="PSUM"))
            ps_z = ctx.enter_context(
                tc.tile_pool(name="ps_z", bufs=3, space="PSUM"))
            ps_cd = ctx.enter_context(
                tc.tile_pool(name="ps_cd", bufs=2, space="PSUM"))

            cs = cs_p.tile([128, n_mats * 128], BF16)
            nc.sync.dma_start(cs[:], cst[:])

            def cmat(i):
                return cs[:, i * 128:(i + 1) * 128]

            LYt = cmat(meta["i_ly"])
            LS4t = cmat(meta["i_ls4"])
            identB = cmat(meta["i_id"])

            for ch in range(NCH):
                # ---- phase A: T1T via W-stationary bf16 matmuls ----
                t1 = t1_p.tile([128, 6 * 48 * O], BF16, tag="t1",
                               name=f"t1_{ch}")
                t1v = t1[:].rearrange("p (cc b t) -> p cc b t", cc=6, b=48)
                for sp in range(NSTR // 2):
                    r0 = (ch * NSTR + 2 * sp) * 128
                    w2 = wch_p.tile([128, 2 * CSH], BF16, tag="w",
                                    name=f"w_{ch}_{sp}")
                    nc.sync.dma_start(
                        w2[:].rearrange("p (s c) -> p s c", s=2),
                        w[r0:r0 + 256, :].rearrange("(s p) c -> p s c",
                                                    p=128))
                    for ccp in range(3):
                        pa = ps_a.tile([128, 512], F32, tag="pa", name="pa")
                        for k in range(4):
                            cc = ccp * 2 + k // 2
                            s01 = k % 2
                            lhsT = w2[:, s01 * CSH + cc * 128:
                                      s01 * CSH + (cc + 1) * 128]
                            nc.tensor.matmul(
                                pa[:, k * 128:(k + 1) * 128], lhsT, LYt)
                        dst = t1v[:, ccp * 2:ccp * 2 + 2,
                                  4 * sp:4 * sp + 4, :]
                        ev.copy(dst, pa[:])
                # ---- phase B: fused mask/symmetrize matmuls ----
                # Z in t-major layout [t, b]: B-evacs land contiguous; the
                # phase-C transpose gathers the (b-pair, t) view via its
                # stationary AP instead.
                zsb = [zsb_p.tile([128, 48 * O], BF16, tag=f"z{cc}",
                                  name=f"z_{ch}_{cc}")
                       for cc in range(6)]
                zbt = [zsb[cc][:].rearrange("p (t b) -> p b t", b=48)
                       for cc in range(6)]
                for g in ginfo:
                    p0 = g["pos0"]
                    if g["is_R"]:
                        h3 = g["half"]
                        w3 = 48 * h3
                        for half in range(2):
                            ccs = [3 * half, 3 * half + 1, 3 * half + 2]
                            zps = {}
                            mvs = {}
                            for cc in ccs:
                                zp = ps_z.tile([128, 512], F32, tag="zp",
                                               name="zp")
                                zps[cc] = zp
                                m_ev = t1v[:, cc, :, p0:p0 + h3].rearrange(
                                    "p b t -> p t b")
                                m_od = t1v[:, cc, :,
                                           p0 + h3:p0 + 2 * h3].rearrange(
                                    "p b t -> p t b")
                                mvs[cc] = (m_ev, m_od)
                                nc.tensor.matmul(zp[:, 0:w3], cmat(g["gi"]),
                                                 m_ev, start=True, stop=False)
                                nc.tensor.matmul(zp[:, w3:2 * w3],
                                                 cmat(g["gi"]),
                                                 m_od, start=False,
                                                 stop=False)
                            for cc in ccs:
                                nc.tensor.matmul(zps[cc][:, 0:w3],
                                                 cmat(g["ji"]), mvs[cc][1],
                                                 start=False, stop=False)
                            for cc in ccs:
                                nc.tensor.matmul(zps[cc][:, w3:2 * w3],
                                                 cmat(g["jni"]), mvs[cc][0],
                                                 start=False, stop=True)
                                dst = zsb[cc][:, p0 * 48:(p0 + 2 * h3) * 48]
                                ev.copy(dst, zps[cc][:, 0:2 * w3])
                    else:
                        sz = g["size"]
                        for half in range(2):
                            ccs = [3 * half, 3 * half + 1, 3 * half + 2]
                            for cc in ccs:
                                zp = ps_z.tile([128, 512], F32, tag="zp",
                                               name="zp")
                                mv = t1v[:, cc, :, p0:p0 + sz].rearrange(
                                    "p b t -> p t b")
                                nc.tensor.matmul(zp[:, 0:48 * sz],
                                                 cmat(g["gi"]), mv)
                                dst = zsb[cc][:, p0 * 48:(p0 + sz) * 48]
                                ev.copy(dst, zp[:, 0:48 * sz])
                # ---- phases C+D per quad ----
                for q in range(NQ):
                    zt = zt_p.tile([128, NSTR * 256], BF16, tag="zt")
                    for jj in range(6):
                        pt = ps_cd.tile([128, 512], F32, tag="pcd",
                                        name="pt")
                        ptb = pt[:].bitcast(BF16)
                        for k in range(8):
                            j = 4 * jj + k // 2
                            cp = k % 2
                            for b01 in range(2):
                                src = zbt[2 * q + cp][:, 2 * j + b01, :]
                                nc.tensor.transpose(
                                    ptb[b01 * 64:(b01 + 1) * 64,
                                        k * 128:(k + 1) * 128], src, identB)
                        ev.copy(zt[:, jj * 1024:(jj + 1) * 1024], ptb[:])
                    for jq in range(0, NSTR, 4):
                        ob = osb_p.tile([128, 1024], BF16, tag="ob")
                        for h2 in range(2):
                            po = ps_cd.tile([128, 512], F32, tag="pcd",
                                            name="po")
                            for k in range(2):
                                j = jq + 2 * h2 + k
                                nc.tensor.matmul(
                                    po[:, k * 256:(k + 1) * 256], LS4t,
                                    zt[:, j * 256:(j + 1) * 256])
                            ev.copy(ob[:, h2 * 512:(h2 + 1) * 512], po[:])
                        nc.sync.dma_start(out[q, ch * 6 + jq // 4], ob[:])
    cap_sync_waits(nc)
    return nc


_CACHE = {}


def prepare(W, U_y, U_x, mask, block_rows, block_cols):
    """Compile (cached) and build per-core input maps."""
    import ml_dtypes
    cst64, ginfo, meta = host_precompute(
        U_y, U_x, mask, block_rows, block_cols)
    cst = cst64.astype(ml_dtypes.bfloat16)
    n_mats = cst.shape[1] // 128
    Wb = np.asarray(W, np.float32).astype(ml_dtypes.bfloat16)

    key = ("nc", n_mats, tuple((g["is_R"], g["pos0"]) for g in ginfo))
    if key not in _CACHE:
        _CACHE[key] = build_kernel(n_mats, ginfo, meta)
    nc = _CACHE[key]

    in_maps = []
    for core in range(NCORE):
        Wsh = np.ascontiguousarray(Wb[:, core * CSH:(core + 1) * CSH])
        in_maps.append({"w": Wsh, "cst": cst})
    return nc, in_maps


def unshard(results):
    outs = []
    for core in range(NCORE):
        o3 = np.asarray(results[core]["out"]).astype(np.float32)
        o = o3.reshape(3, 2, 6, 128, 4, 256).transpose(
            1, 2, 4, 3, 0, 5).reshape(6144, CSH)
        outs.append(o)
    return np.ascontiguousarray(np.concatenate(outs, axis=1))


def kernel(W, U_y, U_x, mask, block_rows, block_cols):
    from concourse import bass_utils
    nc, in_maps = prepare(W, U_y, U_x, mask, block_rows, block_cols)

    res = None
    last_exc = None
    for attempt in range(3):
        try:
            res = bass_utils.run_bass_kernel_spmd(
                nc, in_maps, core_ids=list(range(NCORE)))
            break
        except Exception as e:  # transient NRT_EXEC_UNIT states recover
            last_exc = e
            time.sleep(20 * (attempt + 1))
    if res is None:
        raise last_exc
    return unshard(res.results)
